# revision 19
# baseline (speedup 1.0000x reference)
"""CentroidTripletLoss Trainium2 kernel (8 NeuronCores, feature-dim sharded).

Math (matches the reference):
    centroids[c] = mean of inputs with target c           (segment mean)
    rest[c]      = (sum_c' centroids[c'] - centroids[c]) / (C-1)
    d_ap[b] = ||x_b - centroids[t_b]||,  d_an[b] = ||x_b - rest[t_b]||
    loss = mean(relu(d_ap - d_an + MARGIN))

Distribution: the feature dim D=2048 is sharded 8 ways (256 per core).
Each core computes complete per-class sums for its feature slice (no
centroid all-reduce needed), then per-sample partial squared distances;
a single 64KB AllReduce combines the partials, after which every core
finishes the (tiny) scalar loss reduction redundantly.

Two compiled variants share this builder:
  * sorted_fast: targets are exactly arange(B)//(B//C) (the identity-
    balanced sampler in the reference).  The per-chunk one-hot matrices
    are then compile-time constants (16 distinct patterns), the segment
    sum is one bf16 matmul per chunk, and the per-sample [centroid|rest]
    rows are produced by TensorE from a chunk-major SBUF table
    (diff = E @ [cent|rest] + I @ [-x|-x], accumulated in PSUM), so no
    gather DMA exists at all.
  * general: any targets in [0, C).  fp32 one-hot x 4 class-group
    matmuls, row gather via indirect DMA, VectorE subtracts.
The host picks the variant per call, so arbitrary inputs stay correct.

d_ap - d_an is evaluated as (sap - san) / (sqrt(sap) + sqrt(san)) so the
loose HW sqrt (large ULP budget) only perturbs the result by its own
relative error instead of being amplified by cancellation.
"""

from contextlib import ExitStack

import numpy as np

import concourse.bacc as bacc
import concourse.bass as bass
import concourse.tile as tile
from concourse import mybir
from concourse.bass import IndirectOffsetOnAxis
from concourse.bass_utils import run_bass_kernel_spmd

N_CORES = 8
B = 8192
D = 2048
DS = D // N_CORES  # 256 features per core
C = 512
K = B // C  # 16 samples per class when identity-balanced
NCH = B // 128  # 64 chunks of 128 samples
CG = C // 128  # 4 class groups
CW = DS + 1  # chunk width in resident fp32 X tile (features + ones col)
MARGIN = 0.3

F32 = mybir.dt.float32
BF16 = mybir.dt.bfloat16
I32 = mybir.dt.int32


def _ar(nc, drpool, src_ap, n_cores, stage, name):
    """AllReduce a [128, 64] f32 slab; returns the output DRAM tile."""
    cc_in = drpool.tile([128, NCH], F32, name=f"cc_in_{name}")
    cc_out = drpool.tile([128, NCH], F32, name=f"cc_out_{name}")
    nc.sync.dma_start(cc_in[:], src_ap)
    if stage >= 5:
        nc.gpsimd.collective_compute(
            "AllReduce",
            mybir.AluOpType.add,
            replica_groups=[list(range(n_cores))],
            ins=[cc_in.opt()],
            outs=[cc_out.opt()],
        )
    else:
        nc.sync.dma_start(cc_out[:], cc_in[:])
    return cc_out


def _loss_tail(nc, spool, ppool2, drpool, dbg_dram, out_sb, cc_out1, cc_out2,
               stage):
    """Finish the scalar loss from the two AllReduced interleaved slabs."""
    ones_f = spool.tile([128, 1], F32, tag="ones_f")
    nc.vector.memset(ones_f[:], 1.0)
    sres = spool.tile([128, 2 * NCH], F32, tag="sres")
    nc.sync.dma_start(sres[:, 0:NCH], cc_out1[:])
    nc.sync.dma_start(sres[:, NCH : 2 * NCH], cc_out2[:])
    s3 = sres[:].rearrange("p (c two) -> p c two", two=2)
    sapg = spool.tile([128, NCH], F32, tag="sapg")
    sang = spool.tile([128, NCH], F32, tag="sang")
    nc.vector.tensor_copy(sapg[:], s3[:, :, 0:1])
    nc.vector.tensor_copy(sang[:], s3[:, :, 1:2])
    nc.scalar.dma_start(dbg_dram.ap()[:, 0:NCH], sapg[:])
    nc.scalar.dma_start(dbg_dram.ap()[:, NCH : 2 * NCH], sang[:])
    dapf = spool.tile([128, NCH], F32, tag="dapf")
    danf = spool.tile([128, NCH], F32, tag="danf")
    nc.scalar.sqrt(dapf[:], sapg[:])
    nc.scalar.sqrt(danf[:], sang[:])
    num = spool.tile([128, NCH], F32, tag="num")
    den = spool.tile([128, NCH], F32, tag="den")
    nc.vector.tensor_tensor(num[:], sapg[:], sang[:], op=mybir.AluOpType.subtract)
    nc.vector.tensor_tensor(den[:], dapf[:], danf[:], op=mybir.AluOpType.add)
    rden = spool.tile([128, NCH], F32, tag="rden")
    nc.vector.reciprocal(rden[:], den[:])
    delta = spool.tile([128, NCH], F32, tag="delta")
    nc.vector.tensor_tensor(delta[:], num[:], rden[:], op=mybir.AluOpType.mult)
    terms = spool.tile([128, NCH], F32, tag="terms")
    lcol = spool.tile([128, 1], F32, tag="lcol")
    margin_t = spool.tile([128, 1], F32, tag="margin")
    nc.vector.memset(margin_t[:], MARGIN)
    nc.scalar.activation(
        terms[:],
        delta[:],
        mybir.ActivationFunctionType.Relu,
        bias=margin_t[:, 0:1],
        scale=1.0,
        accum_out=lcol[:, 0:1],
    )
    loss_ps = ppool2.tile([1, 1], F32, tag="loss")
    nc.tensor.matmul(
        loss_ps[:], lhsT=ones_f[:, 0:1], rhs=lcol[:, 0:1], start=True, stop=True
    )
    nc.scalar.mul(out_sb[:], loss_ps[:], 1.0 / B)


def build_sorted(stage=5, n_cores=N_CORES):
    """Fast path: targets == arange(B)//K (verified on host)."""
    nc = bacc.Bacc(None, target_bir_lowering=False, debug=False, num_devices=n_cores)
    # host ships -x in bf16, chunk-major [128, NCH*DS]
    xnb_dram = nc.dram_tensor("xnb", [128, NCH * DS], BF16, kind="ExternalInput")
    loss_dram = nc.dram_tensor("loss", [1, 1], F32, kind="ExternalOutput")
    dbg_dram = nc.dram_tensor("dbg", [128, 2 * NCH], F32, kind="ExternalOutput")

    with tile.TileContext(nc) as tc, ExitStack() as top:
        cpool = top.enter_context(tc.tile_pool(name="const", bufs=1))
        spool = top.enter_context(tc.tile_pool(name="small", bufs=1))
        ppool2 = top.enter_context(tc.tile_pool(name="psum2", bufs=1, space="PSUM"))
        drpool = top.enter_context(tc.tile_pool(name="dram", bufs=1, space="DRAM"))

        # ---------- constants ----------
        # iota row values j (fp32) and helpers, built once
        irow = cpool.tile([128, 128], F32, tag="irow")
        nc.gpsimd.iota(
            irow[:], pattern=[[1, 128]], base=0, channel_multiplier=0,
            allow_small_or_imprecise_dtypes=True,
        )
        pcol_i = cpool.tile([128, 1], I32, tag="pcol_i")
        nc.gpsimd.iota(pcol_i[:], pattern=[[0, 1]], base=0, channel_multiplier=1)
        p16_i = cpool.tile([128, 1], I32, tag="p16_i")
        nc.vector.tensor_scalar(
            p16_i[:], pcol_i[:], 4, None, mybir.AluOpType.arith_shift_right
        )
        p16_f = cpool.tile([128, 1], F32, tag="p16_f")
        nc.vector.tensor_copy(p16_f[:], p16_i[:])
        pcol_f = cpool.tile([128, 1], F32, tag="pcol_f")
        nc.vector.tensor_copy(pcol_f[:], pcol_i[:])
        # I (identity) bf16: I[p, j] = (j == p)
        ident = cpool.tile([128, 128], BF16, tag="ident")
        nc.vector.tensor_scalar(
            ident[:], irow[:], pcol_f[:, 0:1], None, mybir.AluOpType.is_equal
        )
        # A_m bf16 [128, 128]: A_m[p, j] = (j == 8m + p//16), m = chunk % 16
        a_m = []
        for m in range(16):
            t = cpool.tile([128, 1], F32, tag=f"acol{m}", name=f"acol{m}")
            nc.vector.tensor_scalar(
                t[:], p16_f[:], float(8 * m), None, mybir.AluOpType.add
            )
            a = cpool.tile([128, 128], BF16, tag=f"am{m}", name=f"am{m}")
            nc.vector.tensor_scalar(
                a[:], irow[:], t[:, 0:1], None, mybir.AluOpType.is_equal
            )
            a_m.append(a)
        # E128_m bf16 [128, 128]: E128_m[rr, p] = (rr == 8m + p//16)
        # (transpose of A_m: lhsT=E128_m makes out = A_m @ rhs, i.e. the
        # per-sample expansion of the chunk's 8 class rows)
        j16r = cpool.tile([128, 128], F32, tag="j16r")
        nc.gpsimd.iota(
            j16r[:], pattern=[[1, 128]], base=0, channel_multiplier=0,
            allow_small_or_imprecise_dtypes=True,
        )
        j16r_i = cpool.tile([128, 128], I32, tag="j16r_i")
        nc.vector.tensor_copy(j16r_i[:], j16r[:])
        nc.vector.tensor_scalar(
            j16r_i[:], j16r_i[:], 4, None, mybir.AluOpType.arith_shift_right
        )
        j16r_f = cpool.tile([128, 128], F32, tag="j16r_f")
        nc.vector.tensor_copy(j16r_f[:], j16r_i[:])
        e128 = []
        for m in range(16):
            sh = cpool.tile([128, 128], F32, tag=f"e128s{m}", name=f"e128s{m}")
            nc.vector.tensor_scalar(
                sh[:], j16r_f[:], float(8 * m), None, mybir.AluOpType.add
            )
            e = cpool.tile([128, 128], BF16, tag=f"e128{m}", name=f"e128{m}")
            nc.vector.tensor_scalar(
                e[:], sh[:], pcol_f[:, 0:1], None, mybir.AluOpType.is_equal
            )
            e128.append(e)
        ones_col = cpool.tile([128, 1], BF16, tag="ones_col")
        nc.vector.memset(ones_col[:], 1.0)
        ones_row = cpool.tile([1, 128], BF16, tag="ones_row")
        nc.vector.memset(ones_row[:], 1.0)

        # ---------- X load (-x, bf16, 8 x 512KB DMAs, both HWDGE rings) ----------
        xnb = cpool.tile([128, NCH * DS], BF16, tag="xnb")  # holds -x in bf16
        for g in range(8):
            sl = slice(g * 8 * DS, (g + 1) * 8 * DS)
            eng = nc.sync if g % 2 == 0 else nc.scalar
            eng.dma_start(xnb[:, sl], xnb_dram.ap()[:, sl])

        sap = spool.tile([128, NCH], F32, tag="sap")
        san = spool.tile([128, NCH], F32, tag="san")
        out_sb = spool.tile([1, 1], F32, tag="out_sb")

        with ExitStack() as ph1:
            ppool1 = ph1.enter_context(
                tc.tile_pool(name="psum1", bufs=1, space="PSUM")
            )
            # ---------- phase 1: segment sums (of -x, bf16) ----------
            sums_ps = [
                ppool1.tile([128, DS], F32, tag=f"sums{g}", name=f"sums{g}")
                for g in range(CG)
            ]
            for ci in range(NCH):
                g = ci // 16
                nc.tensor.matmul(
                    sums_ps[g][:],
                    lhsT=a_m[ci % 16][:],
                    rhs=xnb[:, ci * DS : (ci + 1) * DS],
                    start=(ci % 16 == 0),
                    stop=(ci % 16 == 15),
                )

            # ---------- phase 1.5: [cent | rest] tiles per class group ----------
            # cent = sums * (-1/K)  (undo the negation)
            tbl = [
                spool.tile([128, 2 * DS], BF16, tag=f"tbl{g}", name=f"tbl{g}")
                for g in range(CG)
            ]
            for g in range(CG):
                nc.vector.tensor_scalar_mul(tbl[g][:, 0:DS], sums_ps[g][:], -1.0 / K)
            tot_ps = ppool1.tile([1, DS], F32, tag="tot")
            for g in range(CG):
                nc.tensor.matmul(
                    tot_ps[:],
                    lhsT=ones_col[:, 0:1],
                    rhs=tbl[g][:, 0:DS],
                    start=(g == 0),
                    stop=(g == CG - 1),
                )
            tot_sb = spool.tile([1, DS], BF16, tag="tot_sb")
            nc.scalar.mul(tot_sb[:], tot_ps[:], 1.0 / (C - 1))
            tb_ps = ppool1.tile([128, DS], F32, tag="tb")
            nc.tensor.matmul(
                tb_ps[:], lhsT=ones_row[:], rhs=tot_sb[:], start=True, stop=True
            )
            resttmp = spool.tile([128, DS], F32, tag="resttmp")
            for g in range(CG):
                nc.vector.tensor_scalar_mul(
                    resttmp[:], tbl[g][:, 0:DS], 1.0 / (C - 1)
                )
                nc.vector.tensor_tensor(
                    tbl[g][:, DS : 2 * DS],
                    tb_ps[:],
                    resttmp[:],
                    op=mybir.AluOpType.subtract,
                )

        if stage >= 3:
            # ---------- phase 2 ----------
            # Per pair of chunks: 4 matmuls build [c-x | rest-x] for both
            # chunks in one 2-bank PSUM tile, one big ACT square (in place),
            # one big DVE reduce -> scol (col 2ci = sap_ci, 2ci+1 = san_ci).
            dfpool = top.enter_context(
                tc.tile_pool(name="diffp", bufs=3, space="PSUM")
            )
            scol = spool.tile([128, 2 * NCH], F32, tag="scol")
            xa = xnb[:]

            def mm1(dpslice, ci):
                g, j = ci // 16, ci % 16
                nc.tensor.matmul(
                    dpslice,
                    lhsT=e128[j][:],
                    rhs=tbl[g][:],
                    start=True,
                    stop=False,
                )

            def mm2(dpslice, ci):
                xrep = bass.AP(
                    xa.tensor,
                    xa.offset + ci * DS,
                    [[xa.ap[0][0], 128], [0, 2], [1, DS]],
                )
                nc.tensor.matmul(
                    dpslice, lhsT=ident[:], rhs=xrep, start=False, stop=True
                )

            cc_out1 = None
            for q in range(NCH // 2):
                c0, c1 = 2 * q, 2 * q + 1
                dp = dfpool.tile([128, 4 * DS], F32, tag="dp", name=f"dp{q}")
                mm1(dp[:, 0 : 2 * DS], c0)
                mm1(dp[:, 2 * DS : 4 * DS], c1)
                mm2(dp[:, 0 : 2 * DS], c0)
                mm2(dp[:, 2 * DS : 4 * DS], c1)
                nc.scalar.activation(
                    dp[:], dp[:], mybir.ActivationFunctionType.Square
                )
                dp3 = dp[:].rearrange("p (h w) -> p h w", w=DS)
                nc.vector.tensor_reduce(
                    scol[:, 4 * q : 4 * q + 4],
                    dp3,
                    op=mybir.AluOpType.add,
                    axis=mybir.AxisListType.X,
                )
                if q == NCH // 4 - 1 and stage >= 4:
                    cc_out1 = _ar(
                        nc, drpool, scol[:, 0:NCH], n_cores, stage, "h1"
                    )

        if stage >= 4:
            cc_out2 = _ar(
                nc, drpool, scol[:, NCH : 2 * NCH], n_cores, stage, "h2"
            )
            _loss_tail(
                nc, spool, ppool2, drpool, dbg_dram, out_sb, cc_out1, cc_out2,
                stage,
            )
        else:
            nc.scalar.mul(out_sb[:], scol[0:1, 0:1], 1.0)
            nc.sync.dma_start(dbg_dram.ap()[:, 0:NCH], scol[:, 0:NCH])
            nc.sync.dma_start(
                dbg_dram.ap()[:, NCH : 2 * NCH], scol[:, NCH : 2 * NCH]
            )

        nc.sync.dma_start(loss_dram.ap(), out_sb[:])

    nc.compile()
    return nc


def build_general(stage=5, n_cores=N_CORES):
    """Correct for arbitrary targets in [0, C)."""
    nc = bacc.Bacc(None, target_bir_lowering=False, debug=False, num_devices=n_cores)
    x_dram = nc.dram_tensor("x", [B, DS], F32, kind="ExternalInput")
    tgt_dram = nc.dram_tensor("tgt", [128, NCH], I32, kind="ExternalInput")
    loss_dram = nc.dram_tensor("loss", [1, 1], F32, kind="ExternalOutput")
    dbg_dram = nc.dram_tensor("dbg", [128, 2 * NCH], F32, kind="ExternalOutput")

    with tile.TileContext(nc) as tc, ExitStack() as top:
        cpool = top.enter_context(tc.tile_pool(name="const", bufs=1))
        ohpool = top.enter_context(tc.tile_pool(name="oh", bufs=4))
        gpool = top.enter_context(tc.tile_pool(name="gath", bufs=4))
        dpool = top.enter_context(tc.tile_pool(name="diff", bufs=3))
        spool = top.enter_context(tc.tile_pool(name="small", bufs=1))
        ppool2 = top.enter_context(tc.tile_pool(name="psum2", bufs=1, space="PSUM"))
        drpool = top.enter_context(tc.tile_pool(name="dram", bufs=1, space="DRAM"))

        xres = cpool.tile([128, NCH * CW], F32, tag="xres")
        x3 = xres[:].rearrange("p (c w) -> p c w", w=CW)
        nc.vector.memset(x3[:, :, DS : DS + 1], 1.0)
        iota_t = cpool.tile([128, C], F32, tag="iota")
        nc.gpsimd.iota(
            iota_t[:], pattern=[[1, C]], base=0, channel_multiplier=0,
            allow_small_or_imprecise_dtypes=True,
        )
        tg32 = cpool.tile([128, NCH], I32, tag="tg32")
        nc.sync.dma_start(tg32[:], tgt_dram.ap())
        tgf = cpool.tile([128, NCH], F32, tag="tgf")
        nc.vector.tensor_copy(tgf[:], tg32[:])
        ones_col = cpool.tile([128, 1], F32, tag="ones_col")
        nc.vector.memset(ones_col[:], 1.0)
        ones_row = cpool.tile([1, 128], F32, tag="ones_row")
        nc.vector.memset(ones_row[:], 1.0)

        xr = x_dram.ap().rearrange("(c p) d -> p c d", p=128)
        for g in range(8):
            nc.sync.dma_start(
                x3[:, g * 8 : (g + 1) * 8, 0:DS], xr[:, g * 8 : (g + 1) * 8, :]
            )

        scol = spool.tile([128, 2 * NCH], F32, tag="scol")
        out_sb = spool.tile([1, 1], F32, tag="out_sb")

        with ExitStack() as ph1:
            ppool1 = ph1.enter_context(
                tc.tile_pool(name="psum1", bufs=1, space="PSUM")
            )
            sums_ps = [
                ppool1.tile([128, CW], F32, tag=f"sums{g}", name=f"sums{g}")
                for g in range(CG)
            ]
            for ci in range(NCH):
                a_t = ohpool.tile([128, C], F32, tag="onehot")
                nc.vector.tensor_scalar(
                    a_t[:],
                    iota_t[:],
                    tgf[:, ci : ci + 1],
                    None,
                    mybir.AluOpType.is_equal,
                )
                rhs = xres[:, ci * CW : (ci + 1) * CW]
                for g in range(CG):
                    nc.tensor.matmul(
                        sums_ps[g][:],
                        lhsT=a_t[:, g * 128 : (g + 1) * 128],
                        rhs=rhs,
                        start=(ci == 0),
                        stop=(ci == NCH - 1),
                    )

            cent = [
                spool.tile([128, DS], F32, tag=f"cent{g}", name=f"cent{g}")
                for g in range(CG)
            ]
            rest = [
                spool.tile([128, DS], F32, tag=f"rest{g}", name=f"rest{g}")
                for g in range(CG)
            ]
            recip = [
                spool.tile([128, 1], F32, tag=f"recip{g}", name=f"recip{g}")
                for g in range(CG)
            ]
            for g in range(CG):
                nc.vector.reciprocal(recip[g][:], sums_ps[g][:, DS : DS + 1])
                nc.vector.tensor_scalar(
                    cent[g][:],
                    sums_ps[g][:, 0:DS],
                    recip[g][:, 0:1],
                    None,
                    mybir.AluOpType.mult,
                )
            tot_ps = ppool1.tile([1, DS], F32, tag="tot")
            for g in range(CG):
                nc.tensor.matmul(
                    tot_ps[:],
                    lhsT=ones_col[:, 0:1],
                    rhs=cent[g][:],
                    start=(g == 0),
                    stop=(g == CG - 1),
                )
            tot_sb = spool.tile([1, DS], F32, tag="tot_sb")
            nc.scalar.mul(tot_sb[:], tot_ps[:], 1.0 / (C - 1))
            tb_ps = ppool1.tile([128, DS], F32, tag="tb")
            nc.tensor.matmul(
                tb_ps[:], lhsT=ones_row[:], rhs=tot_sb[:], start=True, stop=True
            )
            resttmp = spool.tile([128, DS], F32, tag="resttmp")
            for g in range(CG):
                nc.vector.tensor_scalar_mul(resttmp[:], cent[g][:], 1.0 / (C - 1))
                nc.vector.tensor_tensor(
                    rest[g][:], tb_ps[:], resttmp[:], op=mybir.AluOpType.subtract
                )
            table = drpool.tile([C, 2 * DS], F32)
            for g in range(CG):
                nc.sync.dma_start(table[g * 128 : (g + 1) * 128, 0:DS], cent[g][:])
                nc.sync.dma_start(
                    table[g * 128 : (g + 1) * 128, DS : 2 * DS], rest[g][:]
                )

        if stage >= 3:
            for ci in range(NCH):
                cg_t = gpool.tile([128, 2 * DS], F32, tag="gath")
                nc.gpsimd.indirect_dma_start(
                    out=cg_t[:],
                    out_offset=None,
                    in_=table[:],
                    in_offset=IndirectOffsetOnAxis(ap=tg32[:, ci : ci + 1], axis=0),
                )
                xch = xres[:, ci * CW : ci * CW + DS]
                dap = dpool.tile([128, DS], F32, tag="dap")
                dan = dpool.tile([128, DS], F32, tag="dan")
                nc.vector.tensor_tensor(
                    dap[:], xch, cg_t[:, 0:DS], op=mybir.AluOpType.subtract
                )
                nc.vector.tensor_tensor(
                    dan[:], xch, cg_t[:, DS : 2 * DS], op=mybir.AluOpType.subtract
                )
                nc.scalar.activation(
                    dap[:],
                    dap[:],
                    mybir.ActivationFunctionType.Square,
                    accum_out=scol[:, 2 * ci : 2 * ci + 1],
                )
                nc.scalar.activation(
                    dan[:],
                    dan[:],
                    mybir.ActivationFunctionType.Square,
                    accum_out=scol[:, 2 * ci + 1 : 2 * ci + 2],
                )

        if stage >= 4:
            cc_out1 = _ar(nc, drpool, scol[:, 0:NCH], n_cores, stage, "h1")
            cc_out2 = _ar(
                nc, drpool, scol[:, NCH : 2 * NCH], n_cores, stage, "h2"
            )
            _loss_tail(
                nc, spool, ppool2, drpool, dbg_dram, out_sb, cc_out1, cc_out2,
                stage,
            )
        else:
            nc.scalar.mul(out_sb[:], scol[0:1, 0:1], 1.0)
            nc.sync.dma_start(dbg_dram.ap()[:, 0:NCH], scol[:, 0:NCH])
            nc.sync.dma_start(
                dbg_dram.ap()[:, NCH : 2 * NCH], scol[:, NCH : 2 * NCH]
            )

        nc.sync.dma_start(loss_dram.ap(), out_sb[:])

    nc.compile()
    return nc


_PROGRAMS = {}


def _get_program(sorted_fast):
    if sorted_fast not in _PROGRAMS:
        _PROGRAMS[sorted_fast] = (
            build_sorted() if sorted_fast else build_general()
        )
    return _PROGRAMS[sorted_fast]


def is_sorted_balanced(t):
    return bool(np.array_equal(t, np.arange(B, dtype=np.int64) // K))


def make_in_maps(inputs, targets, sorted_fast):
    import ml_dtypes

    x = np.asarray(inputs, dtype=np.float32)
    assert x.shape == (B, D), x.shape
    if sorted_fast:
        # chunk-major negated bf16: xnb[p, ci*DS + d] = -x[ci*128 + p, d0 + d]
        xm = (-x).astype(ml_dtypes.bfloat16)
        maps = []
        for c in range(N_CORES):
            xs = xm[:, c * DS : (c + 1) * DS].reshape(NCH, 128, DS)
            xs = np.ascontiguousarray(xs.transpose(1, 0, 2).reshape(128, NCH * DS))
            maps.append({"xnb": xs})
        return maps
    t = np.asarray(targets).astype(np.int32)
    tgt_re = np.ascontiguousarray(t.reshape(NCH, 128).T)  # [128, NCH]
    return [
        {
            "x": np.ascontiguousarray(x[:, c * DS : (c + 1) * DS]),
            "tgt": tgt_re,
        }
        for c in range(N_CORES)
    ]


def kernel(inputs, targets, num_classes, **_unused):
    assert int(num_classes) == C
    sf = is_sorted_balanced(np.asarray(targets))
    nc = _get_program(sf)
    in_maps = make_in_maps(inputs, targets, sf)
    res = run_bass_kernel_spmd(nc, in_maps, core_ids=list(range(N_CORES)))
    val = np.float32(res.results[0]["loss"][0, 0])
    return np.asarray(val, dtype=np.float32).reshape(())
